# revision 1
# baseline (speedup 1.0000x reference)
"""AttentionPooling (segment softmax-pool) Trainium2 kernel.

Strategy: graphs are sharded across 8 cores (1024 graphs each), nodes follow
their graph (batch is sorted). Each core's 1024 graphs are processed as 8
windows of 128 graphs; the window's nodes are host-padded to a fixed count T.

out[g] = (sum_{n in g} e_n * x_n) / (sum_{n in g} e_n + 1e-8), with
e_n = exp(tanh(x_n @ W1 + b1) @ W2 + b2). The division by the segment sum is
pulled out of the per-node loop, so a single pass over x suffices.

Per 512-node group on device (all matmuls bf16, fp32 accumulate):
  mm1: h^T[hid_out, node] = W1_chunk.T @ x^T (x^T shipped pre-transposed)
  tanh: ACT PSUM->SBUF (bf16 out, per-partition bias b1)
  mm2: logits[node,1] = (h^T chunk as stationary).T @ W2_chunk, accumulated
  exp: one ACT op per group on [128, 4] logits
  S[node, graph] = (iota == batch_rel) * e  -- one fused DVE tensor_scalar
  seg: psum[graph, 0:257] += S.T @ [x | 1]  -- accumulated over the window
Window end: out = psum[:, 0:256] * 1/(psum[:, 256] + 1e-8), DMA to HBM.
"""
import os
import sys

for _p in ("/opt/trn_rl_repo", "/root/.axon_site/_ro/trn_rl_repo"):
    if os.path.isdir(_p) and _p not in sys.path:
        sys.path.insert(0, _p)

import numpy as np
import ml_dtypes

import concourse.bacc as bacc
import concourse.tile as tile
from concourse import mybir
from concourse.bass_utils import run_bass_kernel_spmd

F32 = mybir.dt.float32
BF16 = mybir.dt.bfloat16
BF = ml_dtypes.bfloat16

N_GRAPHS = 8192
HIDDEN = 256
CORES = 8
WPC = 8            # windows per core
WG = 128           # graphs per window
GRP = 512          # nodes per group
EPS = 1e-8

# const block column layout (bf16): W1 blocks (k,m) at 128*(2k+m), k,m in {0,1};
# W2 chunk k at 512+k; iota row at 514:642
C_W1 = 0
C_W2 = 512
C_IOTA = 514
CBW = 642


def _build_program(T: int):
    """Build the SPMD program for window-padded node count T (multiple of 512)."""
    ng = T // GRP          # groups per window
    cols = T // 128        # brel columns per window

    nc = bacc.Bacc("TRN2", target_bir_lowering=False, debug=False,
                   num_devices=CORES)
    xn = nc.dram_tensor("xn", [WPC, T, 258], BF16, kind="ExternalInput").ap()
    xt = nc.dram_tensor("xt", [2, 128, WPC * T], BF16, kind="ExternalInput").ap()
    br = nc.dram_tensor("br", [WPC, 128, cols], F32, kind="ExternalInput").ap()
    cbf = nc.dram_tensor("cbf", [128, CBW], BF16, kind="ExternalInput").ap()
    cf = nc.dram_tensor("cf", [128, 3], F32, kind="ExternalInput").ap()
    out = nc.dram_tensor("out", [WPC * WG, HIDDEN], F32, kind="ExternalOutput").ap()

    with tile.TileContext(nc) as tc:
        with (
            tc.tile_pool(name="const", bufs=1) as cpool,
            tc.tile_pool(name="brp", bufs=2) as brpool,
            tc.tile_pool(name="xnp", bufs=3) as xnpool,
            tc.tile_pool(name="xtp", bufs=3) as xtpool,
            tc.tile_pool(name="htp", bufs=2) as htpool,
            tc.tile_pool(name="etp", bufs=2) as etpool,
            tc.tile_pool(name="sp", bufs=3) as spool,
            tc.tile_pool(name="ow", bufs=2) as owpool,
            tc.tile_pool(name="ph", bufs=2, space="PSUM") as phpool,
            tc.tile_pool(name="pl", bufs=2, space="PSUM") as plpool,
            tc.tile_pool(name="pg", bufs=2, space="PSUM") as pgpool,
        ):
            cb = cpool.tile([128, CBW], BF16)
            cft = cpool.tile([128, 3], F32)
            nc.sync.dma_start(out=cb[:], in_=cbf[:])
            nc.sync.dma_start(out=cft[:], in_=cf[:])
            iota = cb[:, C_IOTA:C_IOTA + 128]

            for w in range(WPC):
                brw = brpool.tile([128, cols], F32)
                nc.sync.dma_start(out=brw[:], in_=br[w])
                pseg = pgpool.tile([128, 257], F32)
                for g in range(ng):
                    # loads
                    xnt = xnpool.tile([128, 4, 258], BF16)
                    nc.sync.dma_start(
                        out=xnt[:],
                        in_=xn[w, g * GRP:(g + 1) * GRP, :].rearrange(
                            "(t p) d -> p t d", p=128))
                    xtt = xtpool.tile([128, 2, GRP], BF16)
                    nc.sync.dma_start(
                        out=xtt[:],
                        in_=xt[:, :, w * T + g * GRP: w * T + (g + 1) * GRP]
                        .rearrange("c p n -> p c n"))

                    # mm1: h^T chunks [128 hid_out, 512 node]
                    ph = phpool.tile([128, 2, GRP], F32)
                    for m in range(2):
                        nc.tensor.matmul(ph[:, m, :],
                                         cb[:, C_W1 + 128 * m:C_W1 + 128 * (m + 1)],
                                         xtt[:, 0, :], start=True, stop=False)
                        nc.tensor.matmul(ph[:, m, :],
                                         cb[:, C_W1 + 128 * (2 + m):C_W1 + 128 * (3 + m)],
                                         xtt[:, 1, :], start=False, stop=True)

                    # tanh -> bf16 h^T in SBUF
                    ht = htpool.tile([128, 2, GRP], BF16)
                    for m in range(2):
                        nc.scalar.activation(ht[:, m, :], ph[:, m, :],
                                             mybir.ActivationFunctionType.Tanh,
                                             bias=cft[:, m:m + 1], scale=1.0)

                    # mm2: logits[128 node, 1] per subtile, accumulate over k
                    pl = plpool.tile([128, 4], F32)
                    for t in range(4):
                        for k in range(2):
                            nc.tensor.matmul(pl[:, t:t + 1],
                                             ht[:, k, 128 * t:128 * (t + 1)],
                                             cb[:, C_W2 + k:C_W2 + k + 1],
                                             start=(k == 0), stop=(k == 1))

                    # exp (adds b2) -> e [128, 4] f32
                    et = etpool.tile([128, 4], F32)
                    nc.scalar.activation(et[:], pl[:],
                                         mybir.ActivationFunctionType.Exp,
                                         bias=cft[:, 2:3], scale=1.0)

                    # scatter matrix + segment matmul per subtile
                    for t in range(4):
                        st = spool.tile([128, 128], BF16)
                        nc.vector.tensor_scalar(st[:], iota,
                                                brw[:, g * 4 + t:g * 4 + t + 1],
                                                et[:, t:t + 1],
                                                op0=mybir.AluOpType.is_equal,
                                                op1=mybir.AluOpType.mult)
                        nc.tensor.matmul(pseg[:], st[:], xnt[:, t, 0:257],
                                         start=(g == 0 and t == 0),
                                         stop=(g == ng - 1 and t == 3))

                # window finalize: out = numer * 1/(denom + eps)
                dtmp = owpool.tile([128, 1], F32)
                nc.vector.tensor_scalar_add(dtmp[:], pseg[:, 256:257], EPS)
                rec = owpool.tile([128, 1], F32)
                nc.vector.reciprocal(rec[:], dtmp[:])
                ow = owpool.tile([128, HIDDEN], F32)
                nc.vector.tensor_scalar(ow[:], pseg[:, 0:256], rec[:], None,
                                        op0=mybir.AluOpType.mult)
                nc.sync.dma_start(out=out[w * WG:(w + 1) * WG, :], in_=ow[:])
    nc.compile()
    return nc


def _prep_inputs(x, batch, W1, b1, W2, b2):
    batch = np.asarray(batch).astype(np.int64)
    x = np.asarray(x, dtype=np.float32)
    n = x.shape[0]

    bnds = np.searchsorted(batch, np.arange(0, N_GRAPHS + 1, WG))
    sizes = np.diff(bnds)
    T = int(max(512, ((int(sizes.max()) + GRP - 1) // GRP) * GRP))
    cols = T // 128

    xbf = x.astype(BF)
    batf = batch.astype(np.float32)

    cbf = np.zeros((128, CBW), dtype=BF)
    W1 = np.asarray(W1, np.float32)
    for k in range(2):
        for m in range(2):
            cbf[:, C_W1 + 128 * (2 * k + m):C_W1 + 128 * (2 * k + m + 1)] = \
                W1[128 * k:128 * (k + 1), 128 * m:128 * (m + 1)].astype(BF)
        cbf[:, C_W2 + k] = np.asarray(W2, np.float32)[128 * k:128 * (k + 1), 0].astype(BF)
    cbf[:, C_IOTA:C_IOTA + 128] = np.tile(np.arange(128, dtype=np.float32), (128, 1)).astype(BF)

    cf = np.zeros((128, 3), dtype=np.float32)
    cf[:, 0] = np.asarray(b1, np.float32)[0:128]
    cf[:, 1] = np.asarray(b1, np.float32)[128:256]
    cf[:, 2] = float(np.asarray(b2, np.float32).reshape(-1)[0])

    in_maps = []
    for c in range(CORES):
        xn_c = np.zeros((WPC, T, 258), dtype=BF)
        xt_c = np.zeros((2, 128, WPC * T), dtype=BF)
        br_c = np.full((WPC, 128, cols), -1.0, dtype=np.float32)
        for w in range(WPC):
            j = c * WPC + w
            s, e = int(bnds[j]), int(bnds[j + 1])
            sz = e - s
            if sz:
                xn_c[w, :sz, 0:256] = xbf[s:e]
                xn_c[w, :sz, 256] = BF(1.0)
                xt_c[0, :, w * T:w * T + sz] = xbf[s:e, 0:128].T
                xt_c[1, :, w * T:w * T + sz] = xbf[s:e, 128:256].T
                tmp = np.full(T, -1.0, dtype=np.float32)
                tmp[:sz] = batf[s:e] - (c * 1024 + w * WG)
                br_c[w] = tmp.reshape(cols, 128).T
        in_maps.append(dict(xn=xn_c, xt=xt_c, br=br_c, cbf=cbf, cf=cf))
    return T, in_maps


_PROGRAM_CACHE = {}


def kernel(x, batch, W1, b1, W2, b2):
    T, in_maps = _prep_inputs(x, batch, W1, b1, W2, b2)
    if T not in _PROGRAM_CACHE:
        _PROGRAM_CACHE[T] = _build_program(T)
    nc = _PROGRAM_CACHE[T]
    res = run_bass_kernel_spmd(nc, in_maps, list(range(CORES))).results
    return np.concatenate([res[c]["out"] for c in range(CORES)], axis=0)


# revision 4
# speedup vs baseline: 268.7640x; 268.7640x over previous
"""AttentionPooling (segment softmax-pool) Trainium2 kernel.

Strategy: graphs are sharded across 8 cores (1024 graphs each), nodes follow
their graph (batch is sorted). Each core's 1024 graphs are processed as 8
windows of 128 graphs; the window's nodes are host-padded to a fixed count T.

out[g] = (sum_{n in g} e_n * x_n) / (sum_{n in g} e_n + 1e-8), with
e_n = exp(tanh(x_n @ W1 + b1) @ W2 + b2). The division by the segment sum is
pulled out of the per-node loop, so a single pass over x suffices.

Per 512-node group on device (all matmuls bf16, fp32 accumulate):
  mm1: h^T[hid_out, node] = W1_chunk.T @ x^T (x^T shipped pre-transposed)
  tanh: ACT PSUM->SBUF (bf16 out, per-partition bias b1)
  mm2: logits[node,1] = (h^T chunk as stationary).T @ W2_chunk, accumulated
  exp: one ACT op per group on [128, 4] logits
  S[node, graph] = (iota == batch_rel) * e  -- one fused DVE tensor_scalar
  seg: psum[graph, 0:257] += S.T @ [x | 1]  -- accumulated over the window
Window end: out = psum[:, 0:256] * 1/(psum[:, 256] + 1e-8), DMA to HBM.
"""
import os
import sys

for _p in ("/opt/trn_rl_repo", "/root/.axon_site/_ro/trn_rl_repo"):
    if os.path.isdir(_p) and _p not in sys.path:
        sys.path.insert(0, _p)

import numpy as np
import ml_dtypes

import concourse.bacc as bacc
import concourse.tile as tile
from concourse import mybir
from concourse.bass_utils import run_bass_kernel_spmd

F32 = mybir.dt.float32
BF16 = mybir.dt.bfloat16
BF = ml_dtypes.bfloat16

N_GRAPHS = 8192
HIDDEN = 256
CORES = 8
WPC = 8            # windows per core
WG = 128           # graphs per window
GRP = 512          # nodes per group
EPS = 1e-8

# const block column layout (bf16): W1 blocks (k,m) at 128*(2k+m), k,m in {0,1};
# W2 chunk k at 512+k; iota row at 514:642
C_W1 = 0
C_W2 = 512
C_IOTA = 514
CBW = 642


def _build_program(T: int, reps: int = 1, variant: str = "full"):
    """Build the SPMD program for window-padded node count T (multiple of 512).

    reps>1 wraps the whole body in a device-side For_i loop for timing.
    variant: "full" | "dma" (loads only) | "nose" (no scatter/seg matmul).
    """
    ng = T // GRP          # groups per window
    cols = T // 128        # brel columns per window

    nc = bacc.Bacc("TRN2", target_bir_lowering=False, debug=False,
                   num_devices=CORES)
    xn = nc.dram_tensor("xn", [WPC, T, 258], BF16, kind="ExternalInput").ap()
    xt = nc.dram_tensor("xt", [2, 128, WPC * T], BF16, kind="ExternalInput").ap()
    br = nc.dram_tensor("br", [WPC, 128, cols], F32, kind="ExternalInput").ap()
    cbf = nc.dram_tensor("cbf", [128, CBW], BF16, kind="ExternalInput").ap()
    cf = nc.dram_tensor("cf", [128, 3], F32, kind="ExternalInput").ap()
    out = nc.dram_tensor("out", [WPC * WG, HIDDEN], F32, kind="ExternalOutput").ap()

    with tile.TileContext(nc) as tc:
        from contextlib import ExitStack
        with ExitStack() as ctx:
            cpool = ctx.enter_context(tc.tile_pool(name="const", bufs=1))
            brpool = ctx.enter_context(tc.tile_pool(name="brp", bufs=2))
            xnpool = ctx.enter_context(tc.tile_pool(name="xnp", bufs=3))
            xtpool = ctx.enter_context(tc.tile_pool(name="xtp", bufs=3))
            htpool = ctx.enter_context(tc.tile_pool(name="htp", bufs=2))
            etpool = ctx.enter_context(tc.tile_pool(name="etp", bufs=2))
            spool = ctx.enter_context(tc.tile_pool(name="sp", bufs=3))
            owpool = ctx.enter_context(tc.tile_pool(name="ow", bufs=2))
            phpool = ctx.enter_context(tc.tile_pool(name="ph", bufs=2, space="PSUM"))
            plpool = ctx.enter_context(tc.tile_pool(name="pl", bufs=2, space="PSUM"))
            pgpool = ctx.enter_context(tc.tile_pool(name="pg", bufs=2, space="PSUM"))
            if reps > 1:
                ctx.enter_context(tc.For_i(0, reps, 1))
            _emit_body(nc, tc, variant, ng, cols,
                       cpool, brpool, xnpool, xtpool, htpool, etpool, spool,
                       owpool, phpool, plpool, pgpool,
                       xn, xt, br, cbf, cf, out, T)
    nc.compile()
    return nc


def _emit_body(nc, tc, variant, ng, cols,
               cpool, brpool, xnpool, xtpool, htpool, etpool, spool,
               owpool, phpool, plpool, pgpool,
               xn, xt, br, cbf, cf, out, T):
    if True:
        if True:
            cb = cpool.tile([128, CBW], BF16)
            cft = cpool.tile([128, 3], F32)
            nc.sync.dma_start(out=cb[:], in_=cbf[:])
            nc.sync.dma_start(out=cft[:], in_=cf[:])
            iota = cb[:, C_IOTA:C_IOTA + 128]

            for w in range(WPC):
                brw = brpool.tile([128, cols], F32)
                nc.sync.dma_start(out=brw[:], in_=br[w])
                pseg = pgpool.tile([128, 257], F32)
                for g in range(ng):
                    # loads
                    xnt = xnpool.tile([128, 4, 258], BF16)
                    nc.sync.dma_start(
                        out=xnt[:],
                        in_=xn[w, g * GRP:(g + 1) * GRP, :].rearrange(
                            "(t p) d -> p t d", p=128))
                    xtt = xtpool.tile([128, 2, GRP], BF16)
                    nc.sync.dma_start(
                        out=xtt[:],
                        in_=xt[:, :, w * T + g * GRP: w * T + (g + 1) * GRP]
                        .rearrange("c p n -> p c n"))

                    if variant == "dma":
                        # consume loads cheaply so they aren't dead
                        dum = etpool.tile([128, 1], F32)
                        nc.vector.tensor_scalar(dum[:], xnt[:, 0, 0:1],
                                                1.0, None, op0=mybir.AluOpType.mult)
                        dum2 = etpool.tile([128, 1], F32)
                        nc.vector.tensor_scalar(dum2[:], xtt[:, 0, 0:1],
                                                1.0, None, op0=mybir.AluOpType.mult)
                        continue

                    # mm1: h^T chunks [128 hid_out, 512 node]
                    ph = phpool.tile([128, 2, GRP], F32)
                    for m in range(2):
                        nc.tensor.matmul(ph[:, m, :],
                                         cb[:, C_W1 + 128 * m:C_W1 + 128 * (m + 1)],
                                         xtt[:, 0, :], start=True, stop=False)
                        nc.tensor.matmul(ph[:, m, :],
                                         cb[:, C_W1 + 128 * (2 + m):C_W1 + 128 * (3 + m)],
                                         xtt[:, 1, :], start=False, stop=True)

                    # tanh -> bf16 h^T in SBUF
                    ht = htpool.tile([128, 2, GRP], BF16)
                    for m in range(2):
                        nc.scalar.activation(ht[:, m, :], ph[:, m, :],
                                             mybir.ActivationFunctionType.Tanh,
                                             bias=cft[:, m:m + 1], scale=1.0)

                    # mm2: logits[128 node, 1] per subtile, accumulate over k
                    pl = plpool.tile([128, 4], F32)
                    for t in range(4):
                        for k in range(2):
                            nc.tensor.matmul(pl[:, t:t + 1],
                                             ht[:, k, 128 * t:128 * (t + 1)],
                                             cb[:, C_W2 + k:C_W2 + k + 1],
                                             start=(k == 0), stop=(k == 1))

                    # exp (adds b2) -> e [128, 4] f32
                    et = etpool.tile([128, 4], F32)
                    nc.scalar.activation(et[:], pl[:],
                                         mybir.ActivationFunctionType.Exp,
                                         bias=cft[:, 2:3], scale=1.0)

                    if variant == "nose":
                        dum3 = etpool.tile([128, 1], F32)
                        nc.vector.tensor_scalar(dum3[:], et[:, 0:1], 1.0, None,
                                                op0=mybir.AluOpType.mult)
                        # still consume xnt so the load stays live
                        dum4 = etpool.tile([128, 1], F32)
                        nc.vector.tensor_scalar(dum4[:], xnt[:, 0, 0:1], 1.0, None,
                                                op0=mybir.AluOpType.mult)
                        continue

                    # scatter matrix + segment matmul per subtile
                    for t in range(4):
                        st = spool.tile([128, 128], BF16)
                        nc.vector.tensor_scalar(st[:], iota,
                                                brw[:, g * 4 + t:g * 4 + t + 1],
                                                et[:, t:t + 1],
                                                op0=mybir.AluOpType.is_equal,
                                                op1=mybir.AluOpType.mult)
                        nc.tensor.matmul(pseg[:], st[:], xnt[:, t, 0:257],
                                         start=(g == 0 and t == 0),
                                         stop=(g == ng - 1 and t == 3))

                if variant != "full":
                    continue
                # window finalize: out = numer * 1/(denom + eps)
                dtmp = owpool.tile([128, 1], F32)
                nc.vector.tensor_scalar_add(dtmp[:], pseg[:, 256:257], EPS)
                rec = owpool.tile([128, 1], F32)
                nc.vector.reciprocal(rec[:], dtmp[:])
                ow = owpool.tile([128, HIDDEN], F32)
                nc.vector.tensor_scalar(ow[:], pseg[:, 0:256], rec[:], None,
                                        op0=mybir.AluOpType.mult)
                nc.sync.dma_start(out=out[w * WG:(w + 1) * WG, :], in_=ow[:])


def _prep_inputs(x, batch, W1, b1, W2, b2):
    batch = np.asarray(batch).astype(np.int64)
    x = np.asarray(x, dtype=np.float32)
    n = x.shape[0]

    bnds = np.searchsorted(batch, np.arange(0, N_GRAPHS + 1, WG))
    sizes = np.diff(bnds)
    T = int(max(512, ((int(sizes.max()) + GRP - 1) // GRP) * GRP))
    cols = T // 128

    xbf = x.astype(BF)
    batf = batch.astype(np.float32)

    cbf = np.zeros((128, CBW), dtype=BF)
    W1 = np.asarray(W1, np.float32)
    for k in range(2):
        for m in range(2):
            cbf[:, C_W1 + 128 * (2 * k + m):C_W1 + 128 * (2 * k + m + 1)] = \
                W1[128 * k:128 * (k + 1), 128 * m:128 * (m + 1)].astype(BF)
        cbf[:, C_W2 + k] = np.asarray(W2, np.float32)[128 * k:128 * (k + 1), 0].astype(BF)
    cbf[:, C_IOTA:C_IOTA + 128] = np.tile(np.arange(128, dtype=np.float32), (128, 1)).astype(BF)

    cf = np.zeros((128, 3), dtype=np.float32)
    cf[:, 0] = np.asarray(b1, np.float32)[0:128]
    cf[:, 1] = np.asarray(b1, np.float32)[128:256]
    cf[:, 2] = float(np.asarray(b2, np.float32).reshape(-1)[0])

    in_maps = []
    for c in range(CORES):
        xn_c = np.zeros((WPC, T, 258), dtype=BF)
        xt_c = np.zeros((2, 128, WPC * T), dtype=BF)
        br_c = np.full((WPC, 128, cols), -1.0, dtype=np.float32)
        for w in range(WPC):
            j = c * WPC + w
            s, e = int(bnds[j]), int(bnds[j + 1])
            sz = e - s
            if sz:
                xn_c[w, :sz, 0:256] = xbf[s:e]
                xn_c[w, :sz, 256] = BF(1.0)
                xt_c[0, :, w * T:w * T + sz] = xbf[s:e, 0:128].T
                xt_c[1, :, w * T:w * T + sz] = xbf[s:e, 128:256].T
                tmp = np.full(T, -1.0, dtype=np.float32)
                tmp[:sz] = batf[s:e] - (c * 1024 + w * WG)
                br_c[w] = tmp.reshape(cols, 128).T
        in_maps.append(dict(xn=xn_c, xt=xt_c, br=br_c, cbf=cbf, cf=cf))
    return T, in_maps


_PROGRAM_CACHE = {}


def kernel(x, batch, W1, b1, W2, b2):
    T, in_maps = _prep_inputs(x, batch, W1, b1, W2, b2)
    if T not in _PROGRAM_CACHE:
        _PROGRAM_CACHE[T] = _build_program(T)
    nc = _PROGRAM_CACHE[T]
    res = run_bass_kernel_spmd(nc, in_maps, list(range(CORES))).results
    return np.concatenate([res[c]["out"] for c in range(CORES)], axis=0)


# revision 5
# speedup vs baseline: 371.7754x; 1.3833x over previous
"""AttentionPooling (segment softmax-pool) Trainium2 kernel.

Graphs are sharded across 8 cores (1024 graphs each); nodes follow their graph
(batch is sorted). Each core's graphs form 8 windows of 128 graphs; a window's
nodes are host-padded to a fixed count T and processed in groups of 512.

out[g] = (sum_{n in g} e_n * x_n) / (sum_{n in g} e_n + 1e-8), with
e_n = exp(tanh(x_n @ W1 + b1) @ W2 + b2); the division is pulled out of the
node loop so one pass over x suffices.

Device pipeline per 512-node group (all matmuls bf16, fp32 accumulate):
  mm1:  h^T[hid_out, node] = W1_chunk.T @ x^T   (x^T shipped pre-transposed)
  tanh: ACT PSUM->SBUF bf16 with per-partition bias b1
  mm2:  logits[node,1] = (h^T chunk as stationary).T @ W2_chunk, k-accumulated
  exp:  one ACT op per group on [128,4] logits (+b2)
  S[node, graph] = (iota == batch_rel) * e    (one fused DVE tensor_scalar)
  seg:  psum[graph, 0:257] += S.T @ [x | 1]   (accumulated over the window)
Window end: out = psum[:,0:256] / (psum[:,256] + eps) -> one DMA.

x is shipped in BOTH layouts (natural rows [x|1|pad] and transposed), host
pre-swizzled so each window is ONE contiguous [128, ng*1032/1024] DMA with
one 16-32KB chunk per partition (line-rate descriptors). The scatter-matmuls
of group g-1 are emitted between group g's mm2 pairs so PE LoadWeights stay
hidden under 257-cycle seg-matmuls.
"""
import os
import sys

for _p in ("/opt/trn_rl_repo", "/root/.axon_site/_ro/trn_rl_repo"):
    if os.path.isdir(_p) and _p not in sys.path:
        sys.path.insert(0, _p)

import numpy as np
import ml_dtypes

import concourse.bacc as bacc
import concourse.tile as tile
from concourse import mybir
from concourse.bass_utils import run_bass_kernel_spmd

F32 = mybir.dt.float32
BF16 = mybir.dt.bfloat16
BF = ml_dtypes.bfloat16

N_GRAPHS = 8192
HIDDEN = 256
CORES = 8
WPC = 8            # windows per core
WG = 128           # graphs per window
GRP = 512          # nodes per group
ROW = 258          # xn row: 256 x + 1.0 + pad
EPS = 1e-8

# const block (bf16): W1 blocks (k,m) at 128*(2k+m); W2 chunk k at 512+k;
# iota row at 514:642
C_W1 = 0
C_W2 = 512
C_IOTA = 514
CBW = 642


def _build_program(T: int, reps: int = 1, variant: str = "full"):
    """variant: "full" | "dma" (loads only) | "nose" (no scatter/seg)."""
    ng = T // GRP
    cols = T // 128
    XNW = ng * 4 * ROW     # xn elems per partition per window
    XTW = ng * 1024        # xt elems per partition per window

    nc = bacc.Bacc("TRN2", target_bir_lowering=False, debug=False,
                   num_devices=CORES)
    xn = nc.dram_tensor("xn", [WPC, 128, XNW], BF16, kind="ExternalInput").ap()
    xt = nc.dram_tensor("xt", [WPC, 128, XTW], BF16, kind="ExternalInput").ap()
    br = nc.dram_tensor("br", [WPC, 128, cols], F32, kind="ExternalInput").ap()
    cbf = nc.dram_tensor("cbf", [128, CBW], BF16, kind="ExternalInput").ap()
    cf = nc.dram_tensor("cf", [128, 3], F32, kind="ExternalInput").ap()
    out = nc.dram_tensor("out", [WPC * WG, HIDDEN], F32, kind="ExternalOutput").ap()

    from contextlib import ExitStack
    with tile.TileContext(nc) as tc:
        with ExitStack() as ctx:
            cpool = ctx.enter_context(tc.tile_pool(name="const", bufs=1))
            brpool = ctx.enter_context(tc.tile_pool(name="brp", bufs=2))
            xnpool = ctx.enter_context(tc.tile_pool(name="xnp", bufs=2))
            xtpool = ctx.enter_context(tc.tile_pool(name="xtp", bufs=2))
            htpool = ctx.enter_context(tc.tile_pool(name="htp", bufs=2))
            etpool = ctx.enter_context(tc.tile_pool(name="etp", bufs=3))
            spool = ctx.enter_context(tc.tile_pool(name="sp", bufs=4))
            owpool = ctx.enter_context(tc.tile_pool(name="ow", bufs=2))
            phpool = ctx.enter_context(tc.tile_pool(name="ph", bufs=2, space="PSUM"))
            plpool = ctx.enter_context(tc.tile_pool(name="pl", bufs=2, space="PSUM"))
            pgpool = ctx.enter_context(tc.tile_pool(name="pg", bufs=2, space="PSUM"))
            if reps > 1:
                ctx.enter_context(tc.For_i(0, reps, 1))

            cb = cpool.tile([128, CBW], BF16)
            cft = cpool.tile([128, 3], F32)
            nc.sync.dma_start(out=cb[:], in_=cbf[:])
            nc.sync.dma_start(out=cft[:], in_=cf[:])
            iota = cb[:, C_IOTA:C_IOTA + 128]

            for w in range(WPC):
                brw = brpool.tile([128, cols], F32)
                nc.sync.dma_start(out=brw[:], in_=br[w])
                xnwt = xnpool.tile([128, XNW], BF16)
                nc.sync.dma_start(out=xnwt[:], in_=xn[w])
                xtwt = xtpool.tile([128, XTW], BF16)
                nc.sync.dma_start(out=xtwt[:], in_=xt[w])

                def xn_view(g, t, n0, n1):
                    base = (g * 4 + t) * ROW
                    return xnwt[:, base + n0:base + n1]

                def xt_view(g, c):
                    base = g * 1024 + c * 512
                    return xtwt[:, base:base + 512]

                if variant == "dma":
                    dum = etpool.tile([128, 1], F32)
                    nc.vector.tensor_scalar(dum[:], xnwt[:, 0:1], 1.0, None,
                                            op0=mybir.AluOpType.mult)
                    dum2 = etpool.tile([128, 1], F32)
                    nc.vector.tensor_scalar(dum2[:], xtwt[:, 0:1], 1.0, None,
                                            op0=mybir.AluOpType.mult)
                    continue

                pseg = pgpool.tile([128, 257], F32)
                prev_et = None

                def emit_seg(gp, t, et_t):
                    st = spool.tile([128, 128], BF16)
                    nc.vector.tensor_scalar(st[:], iota,
                                            brw[:, gp * 4 + t:gp * 4 + t + 1],
                                            et_t,
                                            op0=mybir.AluOpType.is_equal,
                                            op1=mybir.AluOpType.mult)
                    nc.tensor.matmul(pseg[:], st[:], xn_view(gp, t, 0, 257),
                                     start=(gp == 0 and t == 0),
                                     stop=(gp == ng - 1 and t == 3))

                for g in range(ng):
                    # mm1: h^T chunks [128 hid_out, 512 node]
                    ph = phpool.tile([128, 2, GRP], F32)
                    for m in range(2):
                        nc.tensor.matmul(ph[:, m, :],
                                         cb[:, C_W1 + 128 * m:C_W1 + 128 * (m + 1)],
                                         xt_view(g, 0), start=True, stop=False)
                        nc.tensor.matmul(ph[:, m, :],
                                         cb[:, C_W1 + 128 * (2 + m):C_W1 + 128 * (3 + m)],
                                         xt_view(g, 1), start=False, stop=True)

                    ht = htpool.tile([128, 2, GRP], BF16)
                    for m in range(2):
                        nc.scalar.activation(ht[:, m, :], ph[:, m, :],
                                             mybir.ActivationFunctionType.Tanh,
                                             bias=cft[:, m:m + 1], scale=1.0)

                    # mm2 pairs, interleaved with previous group's seg matmuls
                    pl = plpool.tile([128, 4], F32)
                    for t in range(4):
                        for k in range(2):
                            nc.tensor.matmul(pl[:, t:t + 1],
                                             ht[:, k, 128 * t:128 * (t + 1)],
                                             cb[:, C_W2 + k:C_W2 + k + 1],
                                             start=(k == 0), stop=(k == 1))
                        if variant == "full" and prev_et is not None:
                            emit_seg(g - 1, t, prev_et[:, t:t + 1])

                    et = etpool.tile([128, 4], F32)
                    nc.scalar.activation(et[:], pl[:],
                                         mybir.ActivationFunctionType.Exp,
                                         bias=cft[:, 2:3], scale=1.0)

                    if variant == "nose":
                        dum3 = etpool.tile([128, 1], F32)
                        nc.vector.tensor_scalar(dum3[:], et[:, 0:1], 1.0, None,
                                                op0=mybir.AluOpType.mult)
                        dum4 = etpool.tile([128, 1], F32)
                        nc.vector.tensor_scalar(dum4[:], xnwt[:, 0:1], 1.0, None,
                                                op0=mybir.AluOpType.mult)
                        continue
                    prev_et = et

                if variant != "full":
                    continue
                for t in range(4):
                    emit_seg(ng - 1, t, prev_et[:, t:t + 1])

                # window finalize: out = numer * 1/(denom + eps)
                dtmp = owpool.tile([128, 1], F32)
                nc.vector.tensor_scalar_add(dtmp[:], pseg[:, 256:257], EPS)
                rec = owpool.tile([128, 1], F32)
                nc.vector.reciprocal(rec[:], dtmp[:])
                ow = owpool.tile([128, HIDDEN], F32)
                nc.vector.tensor_scalar(ow[:], pseg[:, 0:256], rec[:], None,
                                        op0=mybir.AluOpType.mult)
                nc.sync.dma_start(out=out[w * WG:(w + 1) * WG, :], in_=ow[:])
    nc.compile()
    return nc


def _prep_inputs(x, batch, W1, b1, W2, b2):
    batch = np.asarray(batch).astype(np.int64)
    x = np.asarray(x, dtype=np.float32)

    bnds = np.searchsorted(batch, np.arange(0, N_GRAPHS + 1, WG))
    sizes = np.diff(bnds)
    T = int(max(512, ((int(sizes.max()) + GRP - 1) // GRP) * GRP))
    ng = T // GRP
    cols = T // 128

    xbf = x.astype(BF)
    batf = batch.astype(np.float32)

    cbf = np.zeros((128, CBW), dtype=BF)
    W1 = np.asarray(W1, np.float32)
    for k in range(2):
        for m in range(2):
            cbf[:, C_W1 + 128 * (2 * k + m):C_W1 + 128 * (2 * k + m + 1)] = \
                W1[128 * k:128 * (k + 1), 128 * m:128 * (m + 1)].astype(BF)
        cbf[:, C_W2 + k] = np.asarray(W2, np.float32)[128 * k:128 * (k + 1), 0].astype(BF)
    cbf[:, C_IOTA:C_IOTA + 128] = np.tile(
        np.arange(128, dtype=np.float32), (128, 1)).astype(BF)

    cf = np.zeros((128, 3), dtype=np.float32)
    cf[:, 0] = np.asarray(b1, np.float32)[0:128]
    cf[:, 1] = np.asarray(b1, np.float32)[128:256]
    cf[:, 2] = float(np.asarray(b2, np.float32).reshape(-1)[0])

    in_maps = []
    for c in range(CORES):
        # natural rows [x | 1 | pad] laid out [T, ROW] then swizzled so that
        # partition p holds subtile rows contiguously: [128, ng*4*ROW]
        xn_c = np.zeros((WPC, T, ROW), dtype=BF)
        xt_lin = np.zeros((WPC, 2, 128, T), dtype=BF)
        br_c = np.full((WPC, 128, cols), -1.0, dtype=np.float32)
        for w in range(WPC):
            j = c * WPC + w
            s, e = int(bnds[j]), int(bnds[j + 1])
            sz = e - s
            if sz:
                xn_c[w, :sz, 0:256] = xbf[s:e]
                xn_c[w, :sz, 256] = BF(1.0)
                xt_lin[w, 0, :, :sz] = xbf[s:e, 0:128].T
                xt_lin[w, 1, :, :sz] = xbf[s:e, 128:256].T
                tmp = np.full(T, -1.0, dtype=np.float32)
                tmp[:sz] = batf[s:e] - (c * 1024 + w * WG)
                br_c[w] = tmp.reshape(cols, 128).T
        # xn swizzle: [w, g*512+t*128+p, d] -> [w, p, (g*4+t)*ROW + d]
        xn_sw = np.ascontiguousarray(
            xn_c.reshape(WPC, ng, 4, 128, ROW).transpose(0, 3, 1, 2, 4)
        ).reshape(WPC, 128, ng * 4 * ROW)
        # xt swizzle: [w, c2, p, g*512+n] -> [w, p, (g*2+c2)*512 + n]
        xt_sw = np.ascontiguousarray(
            xt_lin.reshape(WPC, 2, 128, ng, 512).transpose(0, 2, 3, 1, 4)
        ).reshape(WPC, 128, ng * 1024)
        in_maps.append(dict(xn=xn_sw, xt=xt_sw, br=br_c, cbf=cbf, cf=cf))
    return T, in_maps


_PROGRAM_CACHE = {}


def kernel(x, batch, W1, b1, W2, b2):
    T, in_maps = _prep_inputs(x, batch, W1, b1, W2, b2)
    if T not in _PROGRAM_CACHE:
        _PROGRAM_CACHE[T] = _build_program(T)
    nc = _PROGRAM_CACHE[T]
    res = run_bass_kernel_spmd(nc, in_maps, list(range(CORES))).results
    return np.concatenate([res[c]["out"] for c in range(CORES)], axis=0)
